# revision 25
# baseline (speedup 1.0000x reference)
"""Trainium2 Bass kernel for the sum-product "knowledge layer" network.

Computation (see problem reference):
  h0 = encode(x): 8194-row table [-inf, 0, pos0, neg0, pos1, neg1, ...]
       with pos = x (log-probs), neg = log(1 - exp(x)), per batch column.
  4 alternating layers, each: gather rows by ptrs, then segment-reduce over
  contiguous fanin groups (fanin 4 sum-of-logs "product" layers, fanin 2
  logsumexp "sum" layers).

Strategy (2-way batch x 4-way output-group sharding, 8 NeuronCores):
  - The DMA cost model charges per gather descriptor max(bytes, 512)/bus:
    sub-512B descriptors run at half rate, and the gather ISA requires
    256B-multiple rows.  64-col fp32 rows (256B) therefore cost exactly as
    much as 256-col fp16 rows (512B) PER DESCRIPTOR - so we make each
    descriptor carry 4x the batch columns at half precision and shard the
    *output groups* instead of sharding the batch further.
  - Shard the 512 batch columns 2 ways (256 per core, one fp16 table row =
    512B); within a column block, shard the 2048 output groups 4 ways.
    Each core computes only the stage-A (fused layer 0+1) groups its own
    output quarter references: ~2.6K of the 5.9K used groups (the quarters
    are chosen by sorting output groups on their min stage-A reference,
    which clusters shared references and cuts ~11% of the redundancy).
    Group computations are duplicated ~1.8x across the 4 quarters, but
    descriptor count per core drops 4x - net ~2x less DMA time, with zero
    inter-core communication.
  - Per core every table lives in DRAM as [rows, 256] fp16; one row = 512B.
  - Gathers use the SWDGE dma_gather instruction: int16 index list in SBUF,
    each index pulls one 512B row from the DRAM table.
  - DAG pruning (host side, per input set): working back from the core's
    512 output rows, only rows actually referenced downstream are computed.
  - Layer fusion: layers 0+1 fuse into stage A, layers 2+3 into stage B
    (8 gathered leaf rows per group: 2 product quads, summed on DVE, then
    logsumexp'd).
  - Cross-layer software pipelining: stage A's output rows are produced in
    chunk order, A groups are sorted by the max table-0 row they reference,
    and every gather chunk's source AP is narrowed to the exact table
    prefix it needs, so the tile framework only serializes a gather
    against the stores that overlap its prefix.  The encode is likewise
    chunked, with vars laid out partition-major per chunk (var
    1024c + 8p + s at partition p, slot s of chunk c) so each partition
    stores one contiguous 8KB run per chunk and chunk c fills the row
    prefix [2+2048c, 2+2048(c+1)).
  - All four quarters are padded to a common group count and share one
    compiled program (chunk source prefixes take the max over quarters);
    only the index lists and x differ per core.
  - Sum reduction: lse(a,b) = ln(e^a + e^b) computed directly (1 DVE add;
    Exp/Ln on the lightly-loaded ACT engine) - resolve_direct() verifies
    on the host, from the actual x values, that every e^arg stays a
    normal f32, falling back to max + ln(1+exp(min-max)) otherwise.  The
    Exp+Ln activation table is preloaded once (set 6) so the compiler
    inserts no per-instruction table reloads.
  - fp16 tables keep ample accuracy for the 2e-2 gate: host-emulated
    pipeline error is ~1.6e-3 max on the reference data.
"""

import numpy as np

P = 128
B = 256  # batch columns per core (2-way batch shard)
NCOLB = 2  # column blocks
NQ = 4  # output-group quarters per column block
NCORES = NCOLB * NQ
N_VARS = 4096
BATCH = 512
TAB0 = 2 * N_VARS + 2  # 8194
OUT_SIZES = [16384, 8192, 4096, 2048]
FANINS = [4, 2, 4, 2]
FE = 8  # edges per fused group: 2 (sum fanin) x 4 (product fanin)
GPC = 256  # max gather groups per dma_gather chunk (2048 indices)
S_ENC = N_VARS // P  # 32 encode slots per partition
# encode chunk sizes in slots (128 vars each): small head chunks land the
# first t0 prefixes early so gathers start ~4us in; bigger tail chunks
# amortize per-chunk overheads once the pipeline is rolling
SE_LIST = (2, 2, 4, 4, 4, 4, 6, 6)
ENC_CHUNKS = len(SE_LIST)
VAR_BOUNDS = np.cumsum([P * s for s in SE_LIST])  # var-position chunk ends
# ns of serial ACT encode time until chunk c is stored, in units of
# A-group production positions consumed by the gather stream (~91ns per
# 8-desc group at 512B/desc), relative to the head offset; used to decide
# when encode readiness stops constraining the production order
ENC_NS_PER_VAR = 2 * 2 * 0.833  # exp+ln, 2 batch-col elems per var per lane
GROUP_NS = 8 * 22.76 / 16  # gather ns per A group
OUT_PER_Q = OUT_SIZES[3] // NQ  # 512 output groups per quarter


def _joint_greedy(per_q):
    """Order vars, stage-A production, and B production for pipelining.

    Readiness is hierarchical: a B group is gatherable once its 8 stage-A
    rows are stored; an A group once the encode chunks holding its 8 t0
    rows are stored.  With uniform refs, any fixed order leaves nearly
    every group waiting for the last prefix (max-of-8 statistics), which
    serializes each pipeline stage behind the previous one.

    Phase 1 places vars by flat greedy min-new-vars-first over all
    quarters' A groups: maximizes #A groups gatherable per encode-chunk
    prefix (the A gather stream starts earliest and stays supplied).
    Phase 2 orders each quarter's A production by (readiness chunk under
    that placement, first-use by B groups sorted on their readiness
    floor): within what the encode allows, A rows that unlock early B
    groups are produced first, so stage B's chunks need only short tA
    prefixes and overlap stage A's tail.

    Mutates per_q: adds 'a_order' (production order, indexes into used1)
    and 'b_order' (production order, indexes into the quarter's groups).
    Returns (pi, inv): var -> placed position and its inverse.
    """
    import heapq

    # ---- phase 1: var placement (flat A-group greedy) ----
    a_vars = []  # per (q): list of var arrays per A group
    inst = []  # flattened (q, a)
    for q, pq in enumerate(per_q):
        src = pq["a_src0"].reshape(-1, FE)
        avs = [np.unique((g[g >= 2] - 2) // 2) for g in src]
        a_vars.append(avs)
        inst.extend((q, a) for a in range(len(avs)))
    var_insts = [[] for _ in range(N_VARS)]
    for i, (q, a) in enumerate(inst):
        for v in a_vars[q][a]:
            var_insts[v].append(i)
    remaining = [len(a_vars[q][a]) for (q, a) in inst]
    done = [False] * len(inst)
    placed = np.zeros(N_VARS, dtype=bool)
    heap = [(remaining[i], i) for i in range(len(inst))]
    heapq.heapify(heap)
    var_order = []
    while heap:
        r, i = heapq.heappop(heap)
        if done[i] or r != remaining[i]:
            continue
        done[i] = True
        q, a = inst[i]
        for v in a_vars[q][a]:
            if placed[v]:
                continue
            placed[v] = True
            var_order.append(v)
            for j in var_insts[v]:
                if not done[j]:
                    remaining[j] -= 1
                    heapq.heappush(heap, (remaining[j], j))
    var_order.extend(np.nonzero(~placed)[0].tolist())
    inv = np.asarray(var_order, dtype=np.int64)  # position -> original var
    pi = np.empty(N_VARS, dtype=np.int64)  # original var -> position
    pi[inv] = np.arange(N_VARS)

    # ---- phase 2: per-quarter A/B production orders ----
    # B-level set-cover greedy (var-blind): pop the B group with fewest
    # un-activated A refs; the activation order gives each A group a
    # B-driven first-use rank fu.  Production order sorts on
    # t* = max(encode-readiness floor, fu): encode readiness only
    # constrains the early part of the stream (the encode finishes ~halfway
    # through the gather stream), after which A rows are produced in the
    # order stage B wants them, so B chunks need only short tA prefixes.
    enc_done_ns = np.cumsum([P * s * ENC_NS_PER_VAR for s in SE_LIST])
    pmin_c = np.maximum(0.0, (enc_done_ns - 2500.0) / GROUP_NS)
    for q, pq in enumerate(per_q):
        n1 = pq["used1"].size
        avs = a_vars[q]
        loc = np.searchsorted(pq["used1"], pq["bq"]).reshape(OUT_PER_Q, -1)
        b_sets = [np.unique(r) for r in loc]
        a2b = [[] for _ in range(n1)]
        for b, refs in enumerate(b_sets):
            for a in refs:
                a2b[a].append(b)
        rem = np.array([len(s) for s in b_sets], dtype=np.int64)
        act = np.zeros(n1, dtype=bool)
        bdone = np.zeros(OUT_PER_Q, dtype=bool)
        heap2 = [(int(rem[b]), b) for b in range(OUT_PER_Q)]
        heapq.heapify(heap2)
        fu = np.empty(n1, dtype=np.int64)
        rank = 0
        while heap2:
            c, b = heapq.heappop(heap2)
            if bdone[b] or c != rem[b]:
                continue
            bdone[b] = True
            for a in b_sets[b]:
                if not act[a]:
                    act[a] = True
                    fu[a] = rank
                    rank += 1
                    for b2 in a2b[a]:
                        if not bdone[b2]:
                            rem[b2] -= 1
                            heapq.heappush(heap2, (int(rem[b2]), b2))
        rpos = np.array(
            [pi[vs].max() if vs.size else 0 for vs in avs], dtype=np.int64
        )
        rchunk = np.searchsorted(VAR_BOUNDS, rpos, side="right")
        tstar = np.maximum(pmin_c[np.minimum(rchunk, ENC_CHUNKS - 1)], fu)
        pq["a_order"] = np.lexsort((fu, tstar))
        prod_rank = np.empty(n1, dtype=np.int64)
        prod_rank[pq["a_order"]] = np.arange(n1)
        bkey = prod_rank[loc].max(axis=1)
        pq["b_order"] = np.argsort(bkey, kind="stable")
    return pi, inv


def _pad_groups(n):
    return -(-n // P) * P


def _chunk_group_counts(n_groups, tail_ramp):
    """Groups per dma_gather chunk (FE edges per group). Sizes ramp up at
    the start (small first chunk -> quick desc-gen once the source prefix
    lands) and down at the end (short compute tail -> the last store lands
    early and the consumer unblocks sooner)."""
    g = P
    rem = n_groups
    tail = []
    for s in tail_ramp:
        if rem >= s + g:
            tail.append(s)
            rem -= s
    head = []
    for s in (128, 256):
        if rem >= s + g:
            head.append(s)
            rem -= s
    mid = []
    while rem > 0:
        s = min(GPC, rem)
        mid.append(s)
        rem -= s
    return head + mid + tail[::-1]


def plan(ptrs_list):
    """Partition output groups into quarters, prune each quarter's DAG
    bottom-up, fuse layer pairs, readiness-order stage A.

    Returns (stageA, stageB) dicts with common (padded) shape metadata and
    per-quarter index data:
      n_groups: padded group count, common across quarters
      n_src_rows: rows of the gathered table (A: TAB0, B: nA)
      chunks: list of (n_groups_in_chunk, src_prefix_rows); prefixes are
              maxed over quarters so one compiled program serves all cores
      edge_src: per-quarter list of per-edge source rows, production order
    """
    p0, p1, p2, p3 = [np.asarray(p).astype(np.int64) for p in ptrs_list]
    b_l2 = p3.reshape(-1, 2)  # [2048, 2] L2 group ids
    b_src1 = p2.reshape(-1, 4)[b_l2]  # [2048, 2, 4] L1 ids per out group
    # Quarter assignment: sort out-groups by min L1 reference so groups
    # sharing stage-A work land in the same quarter (cuts ~11% of the
    # cross-quarter duplication vs contiguous quarters).
    qorder = np.argsort(b_src1.reshape(OUT_SIZES[3], -1).min(axis=1), kind="stable")
    quarters = [qorder[OUT_PER_Q * q : OUT_PER_Q * (q + 1)] for q in range(NQ)]

    per_q = []
    for q in range(NQ):
        bq = b_src1[quarters[q]]  # [512, 2, 4]
        used1 = np.unique(bq)
        a_l0 = p1.reshape(-1, 2)[used1]  # [n1, 2] L0 group ids
        a_src0 = p0.reshape(-1, 4)[a_l0]  # [n1, 2, 4] t0 rows
        per_q.append({"used1": used1, "a_src0": a_src0, "bq": bq, "ids": quarters[q]})

    # Joint greedy: B pop order -> A activation order -> var placement.
    # Remap t0 rows through the var order (row 2+2u+k of the reference
    # table lands at 2+2*pi[u]+k; rows 0/1 fixed).
    pi, var_inv = _joint_greedy(per_q)
    rowmap = np.empty(TAB0, dtype=np.int64)
    rowmap[0], rowmap[1] = 0, 1
    u = np.arange(N_VARS, dtype=np.int64)
    rowmap[2 + 2 * u] = 2 + 2 * pi
    rowmap[3 + 2 * u] = 3 + 2 * pi
    for pq in per_q:
        pq["a_src0"] = rowmap[pq["a_src0"]]

    nA = max(_pad_groups(pq["used1"].size) for pq in per_q)

    for pq in per_q:
        n1 = pq["used1"].size
        npad = nA - n1
        # padding groups read only row 0: ready before any encode chunk, so
        # putting them first lets chunk 0's desc-gen+gather warm up under
        # the encode instead of idling the DMA engines.  Real groups follow
        # in activation order, so production row t only needs the var
        # prefix placed by the greedy before activating group t.
        srcA = np.zeros((nA, FE), dtype=np.int64)
        srcA[npad:] = pq["a_src0"].reshape(n1, FE)[pq["a_order"]]
        pq["srcA"] = srcA
        prod = np.empty(n1, dtype=np.int64)
        prod[pq["a_order"]] = npad + np.arange(n1)  # A group id -> prod row

        relabel1 = prod[np.searchsorted(pq["used1"], pq["bq"])]  # [512, 2, 4]
        # B rows in pop order: chunk k of stage B then only needs the tA
        # prefix activated by the greedy up to that point.  The host
        # unpermutes output rows after the run.
        pq["srcB"] = relabel1.reshape(-1, FE)[pq["b_order"]]
        prodB = np.empty(OUT_PER_Q, dtype=np.int64)
        prodB[pq["b_order"]] = np.arange(OUT_PER_Q)  # local g -> produced row
        pq["out_prod"] = prodB

    def mk(srcs, n_src_rows, tail_ramp):
        n_groups = srcs[0].shape[0]
        chunks = []
        g_off = 0
        for gc in _chunk_group_counts(n_groups, tail_ramp):
            m = max(int(s[g_off : g_off + gc].max()) + 1 for s in srcs)
            chunks.append((gc, m))
            g_off += gc
        return {
            "n_groups": n_groups,
            "n_src_rows": n_src_rows,
            "chunks": chunks,
            "edge_src": [s.ravel() for s in srcs],
        }

    stages = [
        mk([pq["srcA"] for pq in per_q], TAB0, (128, 128, 128, 256)),
        mk([pq["srcB"] for pq in per_q], nA, (128, 128)),
    ]
    stages[1]["out_prod"] = [pq["out_prod"] for pq in per_q]
    stages[1]["out_ids"] = [pq["ids"] for pq in per_q]
    stages[0]["var_inv"] = var_inv  # x row order for the device table
    stages[0]["direct"] = None
    stages[1]["direct"] = None
    return stages


def resolve_direct(stages, x):
    """Exact host-side bound check for the direct-form lse (e^arg must stay
    a normal f32), evaluated over all quarters."""
    x = np.asarray(x, dtype=np.float64)[stages[0]["var_inv"]]
    pos = x
    neg = np.log1p(-np.exp(x))
    t0 = np.empty((TAB0, x.shape[1]))
    t0[0] = 0.0
    t0[1] = 0.0
    t0[2::2] = pos
    t0[3::2] = neg
    mn_a = 0.0
    mn_b = 0.0
    for q in range(NQ):
        srcA = stages[0]["edge_src"][q].reshape(-1, 2, 4)
        qv = t0[srcA].sum(axis=2)  # [nA, 2, cols]
        m = qv.max(axis=1)
        lo = qv.min(axis=1)
        tA = m + np.log1p(np.exp(lo - m))
        srcB = stages[1]["edge_src"][q].reshape(-1, 2, 4)
        tb = tA[srcB].sum(axis=2)
        mn_a = min(mn_a, qv.min())
        mn_b = min(mn_b, tb.min())
    stages[0]["direct"] = bool(mn_a > -80.0)
    stages[1]["direct"] = bool(mn_b > -80.0)


def reorder_wrap(stage, q):
    """Permute quarter q's per-edge source ids into dma_gather order and
    wrap into the int16 [128, n_edges//16] SBUF layout (position j ->
    [j%16, j//16], replicated across the 8 gpsimd cores' 16-partition
    groups).

    Edge position j of chunk ci maps to partition p=j%128, slot=j//128,
    cc=slot//FE, k=slot%FE, production row = base_ci + p*csub + cc."""
    src = stage["edge_src"][q]
    out = np.empty(stage["n_groups"] * FE, dtype=np.int64)
    base = 0
    e_off = 0
    for gc, _m in stage["chunks"]:
        csub = gc // P
        n_e = gc * FE
        j = np.arange(n_e)
        p = j % P
        slot = j // P
        cc = slot // FE
        k = slot % FE
        row = base + p * csub + cc
        out[e_off : e_off + n_e] = src[row * FE + k]
        base += gc
        e_off += n_e
    assert out.max() < 2**15 and out.min() >= 0
    out = out.astype(np.int16)
    return np.ascontiguousarray(np.tile(out.reshape(-1, 16).T, (8, 1)))


def build_nc(meta):
    """meta: per-stage (n_groups, n_src_rows, chunks-tuple, direct)."""
    import concourse.bacc as bacc
    import concourse.mybir as mybir
    import concourse.tile as tile

    f32 = mybir.dt.float32
    f16 = mybir.dt.float16
    i16 = mybir.dt.int16
    Alu = mybir.AluOpType
    Act = mybir.ActivationFunctionType

    specs = [
        {"n_groups": n, "n_src_rows": s, "chunks": ch, "direct": dr}
        for (n, s, ch, dr) in meta
    ]

    nc = bacc.Bacc("TRN2", target_bir_lowering=False, debug=False)
    x = nc.dram_tensor("x", [P, S_ENC * B], f16, kind="ExternalInput")
    idx_in = [
        nc.dram_tensor(
            f"idx{l}", [P, FE * s["n_groups"] // 16], i16, kind="ExternalInput"
        )
        for l, s in enumerate(specs)
    ]
    out = nc.dram_tensor("out", [OUT_PER_Q, B], f32, kind="ExternalOutput")

    with tile.TileContext(nc) as tc:
        with (
            tc.tile_pool(name="dram", bufs=1, space="DRAM") as dpool,
            tc.tile_pool(name="sb", bufs=4) as gp,
            tc.tile_pool(name="enc", bufs=ENC_CHUNKS) as ep,
            tc.tile_pool(name="hb", bufs=4) as hp,
            tc.tile_pool(name="tmp", bufs=3) as tp,
            tc.tile_pool(name="ix", bufs=1) as ixp,
        ):
            tables = [
                dpool.tile([s["n_src_rows"], B], f16, name=f"t{l}", tag=f"t{l}")
                for l, s in enumerate(specs)
            ]

            # Preload the combined Exp+Ln activation table once; the
            # insert_act_table_loads pass then finds every Exp/Ln already
            # covered and inserts no per-instruction reloads (1283ns each).
            ACT_SET_LN_EXP = 6  # natural_log_exp_and_others
            nc.scalar.add_instruction(
                mybir.InstLoadActFuncSet(
                    name=nc.get_next_instruction_name(),
                    ins=[],
                    outs=[],
                    act_func_set_id=ACT_SET_LN_EXP,
                )
            )

            # table0 rows 0 (-inf in the reference, never gathered) and 1
            # (zeros). Store first so the row prefix [0,2) is ready.
            z = ixp.tile([2, B], f16, tag="z")
            nc.vector.memset(z[:], 0.0)
            nc.sync.dma_start(tables[0][:][0:2, :], z[:])

            # --- encode, chunked: var v sits at partition (v%1024)//SE,
            # slot v%SE of chunk v//1024; pos row 2+2v, neg row 3+2v.
            # Chunk j covers rows [2+2048j, 2+2048(j+1)), a row prefix, so
            # stage-A gather chunks can start before the whole encode
            # finishes.  Within a chunk each partition's 2*SE rows are
            # contiguous, so the fp16 store is 128 runs of 8KB (full-rate
            # DMA; interleaved layouts would pay the sub-512B 2x penalty).
            # All independent loads are emitted before any store so the
            # in-order DMA queue never has a compute-gated store blocking a
            # ready load: x chunks first, then the index lists.
            # Loads and stores interleave on the in-order DMA queue: keep
            # the x loads two chunks ahead of the encode stores so chunk
            # j's store enters the queue early (first gathers depend on it)
            # while later loads still prefetch under the encode compute.
            xv = x[:].rearrange("p (s b) -> p s b", b=B)
            offs = [0]
            for se_j in SE_LIST:
                offs.append(offs[-1] + se_j)
            xls = []

            def load_x(j):
                se_j = SE_LIST[j]
                xl = ep.tile([P, se_j, B], f16, tag="xl")
                nc.sync.dma_start(xl[:], xv[:, offs[j] : offs[j] + se_j, :])
                xls.append(xl)

            load_x(0)
            ix_t = []
            for l, s in enumerate(specs):
                t = ixp.tile([P, FE * s["n_groups"] // 16], i16, tag=f"ix{l}")
                nc.sync.dma_start(t[:], idx_in[l][:])
                ix_t.append(t)
            load_x(1)
            for j, se_j in enumerate(SE_LIST):
                if j + 2 < ENC_CHUNKS:
                    load_x(j + 2)
                xl = xls[j]
                iv = ep.tile([P, se_j, 2, B], f16, tag="enc")
                et = hp.tile([P, se_j, B], f32, tag="h")
                # pos copy rides the idle DVE so ACT only runs exp+ln
                nc.vector.tensor_scalar_add(iv[:][:, :, 0, :], xl[:], 0.0)
                nc.scalar.activation(et[:], xl[:], Act.Exp)
                nc.scalar.activation(
                    iv[:][:, :, 1, :], et[:], Act.Ln, scale=-1.0, bias=1.0
                )
                r0 = 2 + 2 * P * offs[j]
                r1 = 2 + 2 * P * offs[j + 1]
                # row = r0 + 2*se*p + 2*s + k
                nc.sync.dma_start(
                    tables[0][:][r0:r1, :].rearrange(
                        "(p s k) b -> p s k b", s=se_j, k=2
                    ),
                    iv[:],
                )

            # --- fused gather + product-sum + logsumexp stages ---
            for l, s in enumerate(specs):
                last = l + 1 == len(specs)
                dst_tile = out[:] if last else tables[l + 1][:]
                # intermediate tables are fp16 (halves gather descriptor
                # bytes); the final output must stay fp32
                h_dt = f32 if last else f16
                g_off = 0
                e_off = 0
                for gc, m_src in s["chunks"]:
                    csub = gc // P
                    ch = gc * FE
                    S = ch // P
                    g = gp.tile([P, S, B], f16, tag="g")
                    nc.gpsimd.dma_gather(
                        g[:],
                        tables[l][:][0:m_src, :],
                        ix_t[l][:, e_off // 16 : (e_off + ch) // 16],
                        ch,
                        ch,
                        B,
                        single_packet=False,
                    )
                    # [p, group, pair(2), fanin(4), b]
                    v = g[:].rearrange("p (c j k) b -> p c j k b", j=2, k=4)
                    s01 = tp.tile([P, csub, 2, B], f16, tag="m")
                    s23 = tp.tile([P, csub, 2, B], f16, tag="n")
                    ss = gp.tile([P, csub, 2, B], f16, tag="s")
                    nc.vector.tensor_add(s01[:], v[:, :, :, 0, :], v[:, :, :, 1, :])
                    nc.vector.tensor_add(s23[:], v[:, :, :, 2, :], v[:, :, :, 3, :])
                    nc.vector.tensor_add(ss[:], s01[:], s23[:])
                    a = ss[:][:, :, 0, :]
                    b = ss[:][:, :, 1, :]
                    h = hp.tile([P, csub, B], h_dt, tag="h")
                    if s["direct"]:
                        # lse(a,b) = ln(e^a + e^b): host verified e^min is a
                        # normal f32 (no scaling needed). 1 DVE op; the
                        # whole-tile Exp and the Ln ride the ACT engine.
                        e = tp.tile([P, csub, 2, B], f32, tag="d")
                        d = tp.tile([P, csub, B], f32, tag="sp")
                        nc.scalar.activation(e[:], ss[:], Act.Exp)
                        nc.vector.tensor_add(
                            d[:], e[:][:, :, 0, :], e[:][:, :, 1, :]
                        )
                        nc.scalar.activation(h[:], d[:], Act.Ln)
                    else:
                        # wider range: logsumexp = max + ln(1+exp(min-max))
                        m = tp.tile([P, csub, B], f32, tag="d")
                        mn = tp.tile([P, csub, B], f32, tag="sp")
                        sp = tp.tile([P, csub, B], f32, tag="sq")
                        nc.vector.tensor_tensor(m[:], a, b, op=Alu.max)
                        nc.vector.tensor_tensor(mn[:], a, b, op=Alu.min)
                        nc.vector.tensor_tensor(mn[:], mn[:], m[:], op=Alu.subtract)
                        nc.scalar.activation(mn[:], mn[:], Act.Exp)
                        nc.scalar.activation(sp[:], mn[:], Act.Ln, bias=1.0)
                        nc.vector.tensor_add(h[:], m[:], sp[:])
                    # chunk produces rows [g_off, g_off + P*csub):
                    # row = g_off + p*csub + cc
                    nc.sync.dma_start(
                        dst_tile[g_off : g_off + P * csub, :].rearrange(
                            "(p c) b -> p (c b)", p=P
                        ),
                        h[:].rearrange("p c b -> p (c b)"),
                    )
                    g_off += P * csub
                    e_off += ch
    nc.compile()
    return nc


def host_prep(x, ptrs_list, seg_list, stages=None):
    """Host-side sharding + pruning + index preprocessing -> per-core maps."""
    x = np.asarray(x, dtype=np.float32)
    for l, (n_out, f) in enumerate(zip(OUT_SIZES, FANINS)):
        seg = np.asarray(seg_list[l]).astype(np.int64)
        expected = np.repeat(np.arange(n_out, dtype=np.int64), f)
        assert np.array_equal(seg, expected), f"layer {l}: non-uniform segments"

    if stages is None:
        stages = plan(ptrs_list)
    idx_maps = [
        {f"idx{l}": reorder_wrap(s, q) for l, s in enumerate(stages)}
        for q in range(NQ)
    ]

    xvs = []
    xp = x[stages[0]["var_inv"]]  # device var order (greedy placement)
    for cb in range(NCOLB):
        xs = xp[:, cb * B : (cb + 1) * B].astype(np.float16)
        # partition p, slot offs_j+s holds var 128*offs_j + se_j*p + s
        # (partition-major within each encode chunk; see build_nc)
        xv = np.empty((P, S_ENC, B), dtype=np.float16)
        o = 0
        for se_j in SE_LIST:
            xv[:, o : o + se_j] = xs[P * o : P * (o + se_j)].reshape(P, se_j, B)
            o += se_j
        xvs.append(np.ascontiguousarray(xv.reshape(P, -1)))
    # core i -> column block i % NCOLB, quarter i // NCOLB
    return [{"x": xvs[i % NCOLB], **idx_maps[i // NCOLB]} for i in range(NCORES)]


def _meta(stages):
    return tuple(
        (s["n_groups"], s["n_src_rows"], tuple(s["chunks"]), bool(s["direct"]))
        for s in stages
    )


_CACHE = {}


def _get_nc(meta=None):
    if meta is None:
        meta = _CACHE.get("meta")
        assert meta is not None, "call kernel() first"
    if _CACHE.get("meta") != meta:
        _CACHE["nc"] = build_nc(meta)
        _CACHE["meta"] = meta
    return _CACHE["nc"]


def kernel(x, ptrs0, seg0, ptrs1, seg1, ptrs2, seg2, ptrs3, seg3):
    from concourse.bass_utils import run_bass_kernel_spmd

    ptrs_list = [ptrs0, ptrs1, ptrs2, ptrs3]
    stages = plan(ptrs_list)
    resolve_direct(stages, x)
    nc = _get_nc(_meta(stages))
    in_maps = host_prep(x, ptrs_list, [seg0, seg1, seg2, seg3], stages)
    res = run_bass_kernel_spmd(nc, in_maps, core_ids=list(range(NCORES)))
    full = np.empty((OUT_SIZES[3], BATCH), dtype=np.float32)
    for i in range(NCORES):
        cb, q = i % NCOLB, i // NCOLB
        rows = res.results[i]["out"][stages[1]["out_prod"][q]]
        full[stages[1]["out_ids"][q], cb * B : (cb + 1) * B] = rows
    return full


# revision 26
# speedup vs baseline: 1.0015x; 1.0015x over previous
"""Trainium2 Bass kernel for the sum-product "knowledge layer" network.

Computation (see problem reference):
  h0 = encode(x): 8194-row table [-inf, 0, pos0, neg0, pos1, neg1, ...]
       with pos = x (log-probs), neg = log(1 - exp(x)), per batch column.
  4 alternating layers, each: gather rows by ptrs, then segment-reduce over
  contiguous fanin groups (fanin 4 sum-of-logs "product" layers, fanin 2
  logsumexp "sum" layers).

Strategy (2-way batch x 4-way output-group sharding, 8 NeuronCores):
  - The DMA cost model charges per gather descriptor max(bytes, 512)/bus:
    sub-512B descriptors run at half rate, and the gather ISA requires
    256B-multiple rows.  64-col fp32 rows (256B) therefore cost exactly as
    much as 256-col fp16 rows (512B) PER DESCRIPTOR - so we make each
    descriptor carry 4x the batch columns at half precision and shard the
    *output groups* instead of sharding the batch further.
  - Shard the 512 batch columns 2 ways (256 per core, one fp16 table row =
    512B); within a column block, shard the 2048 output groups 4 ways.
    Each core computes only the stage-A (fused layer 0+1) groups its own
    output quarter references: ~2.6K of the 5.9K used groups (the quarters
    are chosen by sorting output groups on their min stage-A reference,
    which clusters shared references and cuts ~11% of the redundancy).
    Group computations are duplicated ~1.8x across the 4 quarters, but
    descriptor count per core drops 4x - net ~2x less DMA time, with zero
    inter-core communication.
  - Per core every table lives in DRAM as [rows, 256] fp16; one row = 512B.
  - Gathers use the SWDGE dma_gather instruction: int16 index list in SBUF,
    each index pulls one 512B row from the DRAM table.
  - DAG pruning (host side, per input set): working back from the core's
    512 output rows, only rows actually referenced downstream are computed.
  - Layer fusion: layers 0+1 fuse into stage A, layers 2+3 into stage B
    (8 gathered leaf rows per group: 2 product quads, summed on DVE, then
    logsumexp'd).
  - Cross-layer software pipelining: stage A's output rows are produced in
    chunk order, A groups are sorted by the max table-0 row they reference,
    and every gather chunk's source AP is narrowed to the exact table
    prefix it needs, so the tile framework only serializes a gather
    against the stores that overlap its prefix.  The encode is likewise
    chunked, with vars laid out partition-major per chunk (var
    1024c + 8p + s at partition p, slot s of chunk c) so each partition
    stores one contiguous 8KB run per chunk and chunk c fills the row
    prefix [2+2048c, 2+2048(c+1)).
  - All four quarters are padded to a common group count and share one
    compiled program (chunk source prefixes take the max over quarters);
    only the index lists and x differ per core.
  - Sum reduction: lse(a,b) = ln(e^a + e^b) computed directly (1 DVE add;
    Exp/Ln on the lightly-loaded ACT engine) - resolve_direct() verifies
    on the host, from the actual x values, that every e^arg stays a
    normal f32, falling back to max + ln(1+exp(min-max)) otherwise.  The
    Exp+Ln activation table is preloaded once (set 6) so the compiler
    inserts no per-instruction table reloads.
  - fp16 tables keep ample accuracy for the 2e-2 gate: host-emulated
    pipeline error is ~1.6e-3 max on the reference data.
"""

import numpy as np

P = 128
B = 256  # batch columns per core (2-way batch shard)
NCOLB = 2  # column blocks
NQ = 4  # output-group quarters per column block
NCORES = NCOLB * NQ
N_VARS = 4096
BATCH = 512
TAB0 = 2 * N_VARS + 2  # 8194
OUT_SIZES = [16384, 8192, 4096, 2048]
FANINS = [4, 2, 4, 2]
FE = 8  # edges per fused group: 2 (sum fanin) x 4 (product fanin)
GPC = 256  # max gather groups per dma_gather chunk (2048 indices)
S_ENC = N_VARS // P  # 32 encode slots per partition
# encode chunk sizes in slots (128 vars each): small head chunks land the
# first t0 prefixes early so gathers start ~4us in; bigger tail chunks
# amortize per-chunk overheads once the pipeline is rolling
SE_LIST = (2, 2, 4, 4, 4, 4, 6, 6)
ENC_CHUNKS = len(SE_LIST)
VAR_BOUNDS = np.cumsum([P * s for s in SE_LIST])  # var-position chunk ends
# ns of serial ACT encode time until chunk c is stored, in units of
# A-group production positions consumed by the gather stream (~91ns per
# 8-desc group at 512B/desc), relative to the head offset; used to decide
# when encode readiness stops constraining the production order
ENC_NS_PER_VAR = 2 * 2 * 0.833  # exp+ln, 2 batch-col elems per var per lane
GROUP_NS = 8 * 22.76 / 16  # gather ns per A group
OUT_PER_Q = OUT_SIZES[3] // NQ  # 512 output groups per quarter


def _joint_greedy(per_q):
    """Order vars, stage-A production, and B production for pipelining.

    Readiness is hierarchical: a B group is gatherable once its 8 stage-A
    rows are stored; an A group once the encode chunks holding its 8 t0
    rows are stored.  With uniform refs, any fixed order leaves nearly
    every group waiting for the last prefix (max-of-8 statistics), which
    serializes each pipeline stage behind the previous one.

    Phase 1 places vars by flat greedy min-new-vars-first over all
    quarters' A groups: maximizes #A groups gatherable per encode-chunk
    prefix (the A gather stream starts earliest and stays supplied).
    Phase 2 orders each quarter's A production by (readiness chunk under
    that placement, first-use by B groups sorted on their readiness
    floor): within what the encode allows, A rows that unlock early B
    groups are produced first, so stage B's chunks need only short tA
    prefixes and overlap stage A's tail.

    Mutates per_q: adds 'a_order' (production order, indexes into used1)
    and 'b_order' (production order, indexes into the quarter's groups).
    Returns (pi, inv): var -> placed position and its inverse.
    """
    import heapq

    # ---- phase 1: var placement (flat A-group greedy) ----
    a_vars = []  # per (q): list of var arrays per A group
    inst = []  # flattened (q, a)
    for q, pq in enumerate(per_q):
        src = pq["a_src0"].reshape(-1, FE)
        avs = [np.unique((g[g >= 2] - 2) // 2) for g in src]
        a_vars.append(avs)
        inst.extend((q, a) for a in range(len(avs)))
    var_insts = [[] for _ in range(N_VARS)]
    for i, (q, a) in enumerate(inst):
        for v in a_vars[q][a]:
            var_insts[v].append(i)
    remaining = [len(a_vars[q][a]) for (q, a) in inst]
    done = [False] * len(inst)
    placed = np.zeros(N_VARS, dtype=bool)
    heap = [(remaining[i], i) for i in range(len(inst))]
    heapq.heapify(heap)
    var_order = []
    while heap:
        r, i = heapq.heappop(heap)
        if done[i] or r != remaining[i]:
            continue
        done[i] = True
        q, a = inst[i]
        for v in a_vars[q][a]:
            if placed[v]:
                continue
            placed[v] = True
            var_order.append(v)
            for j in var_insts[v]:
                if not done[j]:
                    remaining[j] -= 1
                    heapq.heappush(heap, (remaining[j], j))
    var_order.extend(np.nonzero(~placed)[0].tolist())
    inv = np.asarray(var_order, dtype=np.int64)  # position -> original var
    pi = np.empty(N_VARS, dtype=np.int64)  # original var -> position
    pi[inv] = np.arange(N_VARS)

    # ---- phase 2: per-quarter A/B production orders ----
    # B-level set-cover greedy (var-blind): pop the B group with fewest
    # un-activated A refs; the activation order gives each A group a
    # B-driven first-use rank fu.  Production order sorts on
    # t* = max(encode-readiness floor, fu): encode readiness only
    # constrains the early part of the stream (the encode finishes ~halfway
    # through the gather stream), after which A rows are produced in the
    # order stage B wants them, so B chunks need only short tA prefixes.
    enc_done_ns = np.cumsum([P * s * ENC_NS_PER_VAR for s in SE_LIST])
    pmin_c = np.maximum(0.0, (enc_done_ns - 2500.0) / GROUP_NS)
    for q, pq in enumerate(per_q):
        n1 = pq["used1"].size
        avs = a_vars[q]
        loc = np.searchsorted(pq["used1"], pq["bq"]).reshape(OUT_PER_Q, -1)
        b_sets = [np.unique(r) for r in loc]
        a2b = [[] for _ in range(n1)]
        for b, refs in enumerate(b_sets):
            for a in refs:
                a2b[a].append(b)
        rem = np.array([len(s) for s in b_sets], dtype=np.int64)
        act = np.zeros(n1, dtype=bool)
        bdone = np.zeros(OUT_PER_Q, dtype=bool)
        heap2 = [(int(rem[b]), b) for b in range(OUT_PER_Q)]
        heapq.heapify(heap2)
        fu = np.empty(n1, dtype=np.int64)
        rank = 0
        while heap2:
            c, b = heapq.heappop(heap2)
            if bdone[b] or c != rem[b]:
                continue
            bdone[b] = True
            for a in b_sets[b]:
                if not act[a]:
                    act[a] = True
                    fu[a] = rank
                    rank += 1
                    for b2 in a2b[a]:
                        if not bdone[b2]:
                            rem[b2] -= 1
                            heapq.heappush(heap2, (int(rem[b2]), b2))
        rpos = np.array(
            [pi[vs].max() if vs.size else 0 for vs in avs], dtype=np.int64
        )
        rchunk = np.searchsorted(VAR_BOUNDS, rpos, side="right")
        pmin = pmin_c[np.minimum(rchunk, ENC_CHUNKS - 1)]
        # list scheduling: fill production slot t with the encode-feasible
        # (pmin <= t) group stage B wants first; if none is feasible yet,
        # fall back to the next-feasible group.
        bypmin = np.argsort(pmin, kind="stable")
        order = np.empty(n1, dtype=np.int64)
        avail = []
        i = 0
        for t in range(n1):
            while i < n1 and pmin[bypmin[i]] <= t:
                heapq.heappush(avail, (int(fu[bypmin[i]]), int(bypmin[i])))
                i += 1
            if avail:
                order[t] = heapq.heappop(avail)[1]
            else:
                order[t] = bypmin[i]
                i += 1
        pq["a_order"] = order
        prod_rank = np.empty(n1, dtype=np.int64)
        prod_rank[pq["a_order"]] = np.arange(n1)
        bkey = prod_rank[loc].max(axis=1)
        pq["b_order"] = np.argsort(bkey, kind="stable")
    return pi, inv


def _pad_groups(n):
    return -(-n // P) * P


def _chunk_group_counts(n_groups, tail_ramp):
    """Groups per dma_gather chunk (FE edges per group). Sizes ramp up at
    the start (small first chunk -> quick desc-gen once the source prefix
    lands) and down at the end (short compute tail -> the last store lands
    early and the consumer unblocks sooner)."""
    g = P
    rem = n_groups
    tail = []
    for s in tail_ramp:
        if rem >= s + g:
            tail.append(s)
            rem -= s
    head = []
    for s in (128, 256):
        if rem >= s + g:
            head.append(s)
            rem -= s
    mid = []
    while rem > 0:
        s = min(GPC, rem)
        mid.append(s)
        rem -= s
    return head + mid + tail[::-1]


def plan(ptrs_list):
    """Partition output groups into quarters, prune each quarter's DAG
    bottom-up, fuse layer pairs, readiness-order stage A.

    Returns (stageA, stageB) dicts with common (padded) shape metadata and
    per-quarter index data:
      n_groups: padded group count, common across quarters
      n_src_rows: rows of the gathered table (A: TAB0, B: nA)
      chunks: list of (n_groups_in_chunk, src_prefix_rows); prefixes are
              maxed over quarters so one compiled program serves all cores
      edge_src: per-quarter list of per-edge source rows, production order
    """
    p0, p1, p2, p3 = [np.asarray(p).astype(np.int64) for p in ptrs_list]
    b_l2 = p3.reshape(-1, 2)  # [2048, 2] L2 group ids
    b_src1 = p2.reshape(-1, 4)[b_l2]  # [2048, 2, 4] L1 ids per out group
    # Quarter assignment: sort out-groups by min L1 reference so groups
    # sharing stage-A work land in the same quarter (cuts ~11% of the
    # cross-quarter duplication vs contiguous quarters).
    qorder = np.argsort(b_src1.reshape(OUT_SIZES[3], -1).min(axis=1), kind="stable")
    quarters = [qorder[OUT_PER_Q * q : OUT_PER_Q * (q + 1)] for q in range(NQ)]

    per_q = []
    for q in range(NQ):
        bq = b_src1[quarters[q]]  # [512, 2, 4]
        used1 = np.unique(bq)
        a_l0 = p1.reshape(-1, 2)[used1]  # [n1, 2] L0 group ids
        a_src0 = p0.reshape(-1, 4)[a_l0]  # [n1, 2, 4] t0 rows
        per_q.append({"used1": used1, "a_src0": a_src0, "bq": bq, "ids": quarters[q]})

    # Joint greedy: B pop order -> A activation order -> var placement.
    # Remap t0 rows through the var order (row 2+2u+k of the reference
    # table lands at 2+2*pi[u]+k; rows 0/1 fixed).
    pi, var_inv = _joint_greedy(per_q)
    rowmap = np.empty(TAB0, dtype=np.int64)
    rowmap[0], rowmap[1] = 0, 1
    u = np.arange(N_VARS, dtype=np.int64)
    rowmap[2 + 2 * u] = 2 + 2 * pi
    rowmap[3 + 2 * u] = 3 + 2 * pi
    for pq in per_q:
        pq["a_src0"] = rowmap[pq["a_src0"]]

    nA = max(_pad_groups(pq["used1"].size) for pq in per_q)

    for pq in per_q:
        n1 = pq["used1"].size
        npad = nA - n1
        # padding groups read only row 0: ready before any encode chunk, so
        # putting them first lets chunk 0's desc-gen+gather warm up under
        # the encode instead of idling the DMA engines.  Real groups follow
        # in activation order, so production row t only needs the var
        # prefix placed by the greedy before activating group t.
        srcA = np.zeros((nA, FE), dtype=np.int64)
        srcA[npad:] = pq["a_src0"].reshape(n1, FE)[pq["a_order"]]
        pq["srcA"] = srcA
        prod = np.empty(n1, dtype=np.int64)
        prod[pq["a_order"]] = npad + np.arange(n1)  # A group id -> prod row

        relabel1 = prod[np.searchsorted(pq["used1"], pq["bq"])]  # [512, 2, 4]
        # B rows in pop order: chunk k of stage B then only needs the tA
        # prefix activated by the greedy up to that point.  The host
        # unpermutes output rows after the run.
        pq["srcB"] = relabel1.reshape(-1, FE)[pq["b_order"]]
        prodB = np.empty(OUT_PER_Q, dtype=np.int64)
        prodB[pq["b_order"]] = np.arange(OUT_PER_Q)  # local g -> produced row
        pq["out_prod"] = prodB

    def mk(srcs, n_src_rows, tail_ramp):
        n_groups = srcs[0].shape[0]
        chunks = []
        g_off = 0
        for gc in _chunk_group_counts(n_groups, tail_ramp):
            m = max(int(s[g_off : g_off + gc].max()) + 1 for s in srcs)
            chunks.append((gc, m))
            g_off += gc
        return {
            "n_groups": n_groups,
            "n_src_rows": n_src_rows,
            "chunks": chunks,
            "edge_src": [s.ravel() for s in srcs],
        }

    stages = [
        mk([pq["srcA"] for pq in per_q], TAB0, (128, 128, 128, 256)),
        mk([pq["srcB"] for pq in per_q], nA, (128, 128)),
    ]
    stages[1]["out_prod"] = [pq["out_prod"] for pq in per_q]
    stages[1]["out_ids"] = [pq["ids"] for pq in per_q]
    stages[0]["var_inv"] = var_inv  # x row order for the device table
    stages[0]["direct"] = None
    stages[1]["direct"] = None
    return stages


def resolve_direct(stages, x):
    """Exact host-side bound check for the direct-form lse (e^arg must stay
    a normal f32), evaluated over all quarters."""
    x = np.asarray(x, dtype=np.float64)[stages[0]["var_inv"]]
    pos = x
    neg = np.log1p(-np.exp(x))
    t0 = np.empty((TAB0, x.shape[1]))
    t0[0] = 0.0
    t0[1] = 0.0
    t0[2::2] = pos
    t0[3::2] = neg
    mn_a = 0.0
    mn_b = 0.0
    for q in range(NQ):
        srcA = stages[0]["edge_src"][q].reshape(-1, 2, 4)
        qv = t0[srcA].sum(axis=2)  # [nA, 2, cols]
        m = qv.max(axis=1)
        lo = qv.min(axis=1)
        tA = m + np.log1p(np.exp(lo - m))
        srcB = stages[1]["edge_src"][q].reshape(-1, 2, 4)
        tb = tA[srcB].sum(axis=2)
        mn_a = min(mn_a, qv.min())
        mn_b = min(mn_b, tb.min())
    stages[0]["direct"] = bool(mn_a > -80.0)
    stages[1]["direct"] = bool(mn_b > -80.0)


def reorder_wrap(stage, q):
    """Permute quarter q's per-edge source ids into dma_gather order and
    wrap into the int16 [128, n_edges//16] SBUF layout (position j ->
    [j%16, j//16], replicated across the 8 gpsimd cores' 16-partition
    groups).

    Edge position j of chunk ci maps to partition p=j%128, slot=j//128,
    cc=slot//FE, k=slot%FE, production row = base_ci + p*csub + cc."""
    src = stage["edge_src"][q]
    out = np.empty(stage["n_groups"] * FE, dtype=np.int64)
    base = 0
    e_off = 0
    for gc, _m in stage["chunks"]:
        csub = gc // P
        n_e = gc * FE
        j = np.arange(n_e)
        p = j % P
        slot = j // P
        cc = slot // FE
        k = slot % FE
        row = base + p * csub + cc
        out[e_off : e_off + n_e] = src[row * FE + k]
        base += gc
        e_off += n_e
    assert out.max() < 2**15 and out.min() >= 0
    out = out.astype(np.int16)
    return np.ascontiguousarray(np.tile(out.reshape(-1, 16).T, (8, 1)))


def build_nc(meta):
    """meta: per-stage (n_groups, n_src_rows, chunks-tuple, direct)."""
    import concourse.bacc as bacc
    import concourse.mybir as mybir
    import concourse.tile as tile

    f32 = mybir.dt.float32
    f16 = mybir.dt.float16
    i16 = mybir.dt.int16
    Alu = mybir.AluOpType
    Act = mybir.ActivationFunctionType

    specs = [
        {"n_groups": n, "n_src_rows": s, "chunks": ch, "direct": dr}
        for (n, s, ch, dr) in meta
    ]

    nc = bacc.Bacc("TRN2", target_bir_lowering=False, debug=False)
    x = nc.dram_tensor("x", [P, S_ENC * B], f16, kind="ExternalInput")
    idx_in = [
        nc.dram_tensor(
            f"idx{l}", [P, FE * s["n_groups"] // 16], i16, kind="ExternalInput"
        )
        for l, s in enumerate(specs)
    ]
    out = nc.dram_tensor("out", [OUT_PER_Q, B], f32, kind="ExternalOutput")

    with tile.TileContext(nc) as tc:
        with (
            tc.tile_pool(name="dram", bufs=1, space="DRAM") as dpool,
            tc.tile_pool(name="sb", bufs=4) as gp,
            tc.tile_pool(name="enc", bufs=ENC_CHUNKS) as ep,
            tc.tile_pool(name="hb", bufs=4) as hp,
            tc.tile_pool(name="tmp", bufs=3) as tp,
            tc.tile_pool(name="ix", bufs=1) as ixp,
        ):
            tables = [
                dpool.tile([s["n_src_rows"], B], f16, name=f"t{l}", tag=f"t{l}")
                for l, s in enumerate(specs)
            ]

            # Preload the combined Exp+Ln activation table once; the
            # insert_act_table_loads pass then finds every Exp/Ln already
            # covered and inserts no per-instruction reloads (1283ns each).
            ACT_SET_LN_EXP = 6  # natural_log_exp_and_others
            nc.scalar.add_instruction(
                mybir.InstLoadActFuncSet(
                    name=nc.get_next_instruction_name(),
                    ins=[],
                    outs=[],
                    act_func_set_id=ACT_SET_LN_EXP,
                )
            )

            # table0 rows 0 (-inf in the reference, never gathered) and 1
            # (zeros). Store first so the row prefix [0,2) is ready.
            z = ixp.tile([2, B], f16, tag="z")
            nc.vector.memset(z[:], 0.0)
            nc.sync.dma_start(tables[0][:][0:2, :], z[:])

            # --- encode, chunked: var v sits at partition (v%1024)//SE,
            # slot v%SE of chunk v//1024; pos row 2+2v, neg row 3+2v.
            # Chunk j covers rows [2+2048j, 2+2048(j+1)), a row prefix, so
            # stage-A gather chunks can start before the whole encode
            # finishes.  Within a chunk each partition's 2*SE rows are
            # contiguous, so the fp16 store is 128 runs of 8KB (full-rate
            # DMA; interleaved layouts would pay the sub-512B 2x penalty).
            # All independent loads are emitted before any store so the
            # in-order DMA queue never has a compute-gated store blocking a
            # ready load: x chunks first, then the index lists.
            # Loads and stores interleave on the in-order DMA queue: keep
            # the x loads two chunks ahead of the encode stores so chunk
            # j's store enters the queue early (first gathers depend on it)
            # while later loads still prefetch under the encode compute.
            xv = x[:].rearrange("p (s b) -> p s b", b=B)
            offs = [0]
            for se_j in SE_LIST:
                offs.append(offs[-1] + se_j)
            xls = []

            def load_x(j):
                se_j = SE_LIST[j]
                xl = ep.tile([P, se_j, B], f16, tag="xl")
                nc.sync.dma_start(xl[:], xv[:, offs[j] : offs[j] + se_j, :])
                xls.append(xl)

            load_x(0)
            ix_t = []
            for l, s in enumerate(specs):
                t = ixp.tile([P, FE * s["n_groups"] // 16], i16, tag=f"ix{l}")
                nc.sync.dma_start(t[:], idx_in[l][:])
                ix_t.append(t)
            load_x(1)
            for j, se_j in enumerate(SE_LIST):
                if j + 2 < ENC_CHUNKS:
                    load_x(j + 2)
                xl = xls[j]
                iv = ep.tile([P, se_j, 2, B], f16, tag="enc")
                et = hp.tile([P, se_j, B], f32, tag="h")
                # pos copy rides the idle DVE so ACT only runs exp+ln
                nc.vector.tensor_scalar_add(iv[:][:, :, 0, :], xl[:], 0.0)
                nc.scalar.activation(et[:], xl[:], Act.Exp)
                nc.scalar.activation(
                    iv[:][:, :, 1, :], et[:], Act.Ln, scale=-1.0, bias=1.0
                )
                r0 = 2 + 2 * P * offs[j]
                r1 = 2 + 2 * P * offs[j + 1]
                # row = r0 + 2*se*p + 2*s + k
                nc.sync.dma_start(
                    tables[0][:][r0:r1, :].rearrange(
                        "(p s k) b -> p s k b", s=se_j, k=2
                    ),
                    iv[:],
                )

            # --- fused gather + product-sum + logsumexp stages ---
            for l, s in enumerate(specs):
                last = l + 1 == len(specs)
                dst_tile = out[:] if last else tables[l + 1][:]
                # intermediate tables are fp16 (halves gather descriptor
                # bytes); the final output must stay fp32
                h_dt = f32 if last else f16
                g_off = 0
                e_off = 0
                for gc, m_src in s["chunks"]:
                    csub = gc // P
                    ch = gc * FE
                    S = ch // P
                    g = gp.tile([P, S, B], f16, tag="g")
                    nc.gpsimd.dma_gather(
                        g[:],
                        tables[l][:][0:m_src, :],
                        ix_t[l][:, e_off // 16 : (e_off + ch) // 16],
                        ch,
                        ch,
                        B,
                        single_packet=False,
                    )
                    # [p, group, pair(2), fanin(4), b]
                    v = g[:].rearrange("p (c j k) b -> p c j k b", j=2, k=4)
                    s01 = tp.tile([P, csub, 2, B], f16, tag="m")
                    s23 = tp.tile([P, csub, 2, B], f16, tag="n")
                    ss = gp.tile([P, csub, 2, B], f16, tag="s")
                    nc.vector.tensor_add(s01[:], v[:, :, :, 0, :], v[:, :, :, 1, :])
                    nc.vector.tensor_add(s23[:], v[:, :, :, 2, :], v[:, :, :, 3, :])
                    nc.vector.tensor_add(ss[:], s01[:], s23[:])
                    a = ss[:][:, :, 0, :]
                    b = ss[:][:, :, 1, :]
                    h = hp.tile([P, csub, B], h_dt, tag="h")
                    if s["direct"]:
                        # lse(a,b) = ln(e^a + e^b): host verified e^min is a
                        # normal f32 (no scaling needed). 1 DVE op; the
                        # whole-tile Exp and the Ln ride the ACT engine.
                        e = tp.tile([P, csub, 2, B], f32, tag="d")
                        d = tp.tile([P, csub, B], f32, tag="sp")
                        nc.scalar.activation(e[:], ss[:], Act.Exp)
                        nc.vector.tensor_add(
                            d[:], e[:][:, :, 0, :], e[:][:, :, 1, :]
                        )
                        nc.scalar.activation(h[:], d[:], Act.Ln)
                    else:
                        # wider range: logsumexp = max + ln(1+exp(min-max))
                        m = tp.tile([P, csub, B], f32, tag="d")
                        mn = tp.tile([P, csub, B], f32, tag="sp")
                        sp = tp.tile([P, csub, B], f32, tag="sq")
                        nc.vector.tensor_tensor(m[:], a, b, op=Alu.max)
                        nc.vector.tensor_tensor(mn[:], a, b, op=Alu.min)
                        nc.vector.tensor_tensor(mn[:], mn[:], m[:], op=Alu.subtract)
                        nc.scalar.activation(mn[:], mn[:], Act.Exp)
                        nc.scalar.activation(sp[:], mn[:], Act.Ln, bias=1.0)
                        nc.vector.tensor_add(h[:], m[:], sp[:])
                    # chunk produces rows [g_off, g_off + P*csub):
                    # row = g_off + p*csub + cc
                    nc.sync.dma_start(
                        dst_tile[g_off : g_off + P * csub, :].rearrange(
                            "(p c) b -> p (c b)", p=P
                        ),
                        h[:].rearrange("p c b -> p (c b)"),
                    )
                    g_off += P * csub
                    e_off += ch
    nc.compile()
    return nc


def host_prep(x, ptrs_list, seg_list, stages=None):
    """Host-side sharding + pruning + index preprocessing -> per-core maps."""
    x = np.asarray(x, dtype=np.float32)
    for l, (n_out, f) in enumerate(zip(OUT_SIZES, FANINS)):
        seg = np.asarray(seg_list[l]).astype(np.int64)
        expected = np.repeat(np.arange(n_out, dtype=np.int64), f)
        assert np.array_equal(seg, expected), f"layer {l}: non-uniform segments"

    if stages is None:
        stages = plan(ptrs_list)
    idx_maps = [
        {f"idx{l}": reorder_wrap(s, q) for l, s in enumerate(stages)}
        for q in range(NQ)
    ]

    xvs = []
    xp = x[stages[0]["var_inv"]]  # device var order (greedy placement)
    for cb in range(NCOLB):
        xs = xp[:, cb * B : (cb + 1) * B].astype(np.float16)
        # partition p, slot offs_j+s holds var 128*offs_j + se_j*p + s
        # (partition-major within each encode chunk; see build_nc)
        xv = np.empty((P, S_ENC, B), dtype=np.float16)
        o = 0
        for se_j in SE_LIST:
            xv[:, o : o + se_j] = xs[P * o : P * (o + se_j)].reshape(P, se_j, B)
            o += se_j
        xvs.append(np.ascontiguousarray(xv.reshape(P, -1)))
    # core i -> column block i % NCOLB, quarter i // NCOLB
    return [{"x": xvs[i % NCOLB], **idx_maps[i // NCOLB]} for i in range(NCORES)]


def _meta(stages):
    return tuple(
        (s["n_groups"], s["n_src_rows"], tuple(s["chunks"]), bool(s["direct"]))
        for s in stages
    )


_CACHE = {}


def _get_nc(meta=None):
    if meta is None:
        meta = _CACHE.get("meta")
        assert meta is not None, "call kernel() first"
    if _CACHE.get("meta") != meta:
        _CACHE["nc"] = build_nc(meta)
        _CACHE["meta"] = meta
    return _CACHE["nc"]


def kernel(x, ptrs0, seg0, ptrs1, seg1, ptrs2, seg2, ptrs3, seg3):
    from concourse.bass_utils import run_bass_kernel_spmd

    ptrs_list = [ptrs0, ptrs1, ptrs2, ptrs3]
    stages = plan(ptrs_list)
    resolve_direct(stages, x)
    nc = _get_nc(_meta(stages))
    in_maps = host_prep(x, ptrs_list, [seg0, seg1, seg2, seg3], stages)
    res = run_bass_kernel_spmd(nc, in_maps, core_ids=list(range(NCORES)))
    full = np.empty((OUT_SIZES[3], BATCH), dtype=np.float32)
    for i in range(NCORES):
        cb, q = i % NCOLB, i // NCOLB
        rows = res.results[i]["out"][stages[1]["out_prod"][q]]
        full[stages[1]["out_ids"][q], cb * B : (cb + 1) * B] = rows
    return full


# revision 32
# speedup vs baseline: 1.0272x; 1.0257x over previous
"""Trainium2 Bass kernel for the sum-product "knowledge layer" network.

Computation (see problem reference):
  h0 = encode(x): 8194-row table [-inf, 0, pos0, neg0, pos1, neg1, ...]
       with pos = x (log-probs), neg = log(1 - exp(x)), per batch column.
  4 alternating layers, each: gather rows by ptrs, then segment-reduce over
  contiguous fanin groups (fanin 4 sum-of-logs "product" layers, fanin 2
  logsumexp "sum" layers).

Strategy (2-way batch x 4-way output-group sharding, 8 NeuronCores):
  - The DMA cost model charges per gather descriptor max(bytes, 512)/bus:
    sub-512B descriptors run at half rate, and the gather ISA requires
    256B-multiple rows.  64-col fp32 rows (256B) therefore cost exactly as
    much as 256-col fp16 rows (512B) PER DESCRIPTOR - so we make each
    descriptor carry 4x the batch columns at half precision and shard the
    *output groups* instead of sharding the batch further.
  - Shard the 512 batch columns 2 ways (256 per core, one fp16 table row =
    512B); within a column block, shard the 2048 output groups 4 ways.
    Each core computes only the stage-A (fused layer 0+1) groups its own
    output quarter references: ~2.6K of the 5.9K used groups (the quarters
    are chosen by sorting output groups on their min stage-A reference,
    which clusters shared references and cuts ~11% of the redundancy).
    Group computations are duplicated ~1.8x across the 4 quarters, but
    descriptor count per core drops 4x - net ~2x less DMA time, with zero
    inter-core communication.
  - Per core every table lives in DRAM as [rows, 256] fp16; one row = 512B.
  - Gathers use the SWDGE dma_gather instruction: int16 index list in SBUF,
    each index pulls one 512B row from the DRAM table.
  - DAG pruning (host side, per input set): working back from the core's
    512 output rows, only rows actually referenced downstream are computed.
  - Layer fusion: layers 0+1 fuse into stage A, layers 2+3 into stage B
    (8 gathered leaf rows per group: 2 product quads, summed on DVE, then
    logsumexp'd).
  - Cross-layer software pipelining: stage A's output rows are produced in
    chunk order, A groups are sorted by the max table-0 row they reference,
    and every gather chunk's source AP is narrowed to the exact table
    prefix it needs, so the tile framework only serializes a gather
    against the stores that overlap its prefix.  The encode is likewise
    chunked, with vars laid out partition-major per chunk (var
    1024c + 8p + s at partition p, slot s of chunk c) so each partition
    stores one contiguous 8KB run per chunk and chunk c fills the row
    prefix [2+2048c, 2+2048(c+1)).
  - All four quarters are padded to a common group count and share one
    compiled program (chunk source prefixes take the max over quarters);
    only the index lists and x differ per core.
  - Sum reduction: lse(a,b) = ln(e^a + e^b) computed directly (1 DVE add;
    Exp/Ln on the lightly-loaded ACT engine) - resolve_direct() verifies
    on the host, from the actual x values, that every e^arg stays a
    normal f32, falling back to max + ln(1+exp(min-max)) otherwise.  The
    Exp+Ln activation table is preloaded once (set 6) so the compiler
    inserts no per-instruction table reloads.
  - fp16 tables keep ample accuracy for the 2e-2 gate: host-emulated
    pipeline error is ~1.6e-3 max on the reference data.
"""

import numpy as np

P = 128
B = 256  # batch columns per core (2-way batch shard)
NCOLB = 2  # column blocks
NQ = 4  # output-group quarters per column block
NCORES = NCOLB * NQ
N_VARS = 4096
BATCH = 512
TAB0 = 2 * N_VARS + 2  # 8194
OUT_SIZES = [16384, 8192, 4096, 2048]
FANINS = [4, 2, 4, 2]
FE = 8  # edges per fused group: 2 (sum fanin) x 4 (product fanin)
GPC = 256  # max gather groups per dma_gather chunk (2048 indices)
S_ENC = N_VARS // P  # 32 encode slots per partition
# encode chunk sizes in slots (128 vars each): small head chunks land the
# first t0 prefixes early so gathers start ~4us in; bigger tail chunks
# amortize per-chunk overheads once the pipeline is rolling
SE_LIST = (2, 2, 4, 4, 4, 4, 6, 6)
ENC_CHUNKS = len(SE_LIST)
VAR_BOUNDS = np.cumsum([P * s for s in SE_LIST])  # var-position chunk ends
# ns of serial ACT encode time until chunk c is stored, in units of
# A-group production positions consumed by the gather stream (~91ns per
# 8-desc group at 512B/desc), relative to the head offset; used to decide
# when encode readiness stops constraining the production order
ENC_NS_PER_VAR = 2 * 2 * 0.833  # exp+ln, 2 batch-col elems per var per lane
GROUP_NS = 8 * 22.76 / 16  # gather ns per A group
OUT_PER_Q = OUT_SIZES[3] // NQ  # 512 output groups per quarter


def _joint_greedy(per_q):
    """Order vars, stage-A production, and B production for pipelining.

    Readiness is hierarchical: a B group is gatherable once its 8 stage-A
    rows are stored; an A group once the encode chunks holding its 8 t0
    rows are stored.  With uniform refs, any fixed order leaves nearly
    every group waiting for the last prefix (max-of-8 statistics), which
    serializes each pipeline stage behind the previous one.

    Phase 1 places vars by flat greedy min-new-vars-first over all
    quarters' A groups: maximizes #A groups gatherable per encode-chunk
    prefix (the A gather stream starts earliest and stays supplied).
    Phase 2 orders each quarter's A production by (readiness chunk under
    that placement, first-use by B groups sorted on their readiness
    floor): within what the encode allows, A rows that unlock early B
    groups are produced first, so stage B's chunks need only short tA
    prefixes and overlap stage A's tail.

    Mutates per_q: adds 'a_order' (production order, indexes into used1)
    and 'b_order' (production order, indexes into the quarter's groups).
    Returns (pi, inv): var -> placed position and its inverse.
    """
    import heapq

    # ---- phase 1: var placement (flat A-group greedy) ----
    a_vars = []  # per (q): list of var arrays per A group
    inst = []  # flattened (q, a)
    for q, pq in enumerate(per_q):
        src = pq["a_src0"].reshape(-1, FE)
        avs = [np.unique((g[g >= 2] - 2) // 2) for g in src]
        a_vars.append(avs)
        inst.extend((q, a) for a in range(len(avs)))
    var_insts = [[] for _ in range(N_VARS)]
    for i, (q, a) in enumerate(inst):
        for v in a_vars[q][a]:
            var_insts[v].append(i)
    remaining = [len(a_vars[q][a]) for (q, a) in inst]
    done = [False] * len(inst)
    placed = np.zeros(N_VARS, dtype=bool)
    heap = [(remaining[i], i) for i in range(len(inst))]
    heapq.heapify(heap)
    var_order = []
    while heap:
        r, i = heapq.heappop(heap)
        if done[i] or r != remaining[i]:
            continue
        done[i] = True
        q, a = inst[i]
        for v in a_vars[q][a]:
            if placed[v]:
                continue
            placed[v] = True
            var_order.append(v)
            for j in var_insts[v]:
                if not done[j]:
                    remaining[j] -= 1
                    heapq.heappush(heap, (remaining[j], j))
    var_order.extend(np.nonzero(~placed)[0].tolist())
    inv = np.asarray(var_order, dtype=np.int64)  # position -> original var
    pi = np.empty(N_VARS, dtype=np.int64)  # original var -> position
    pi[inv] = np.arange(N_VARS)

    # ---- phase 2: per-quarter A/B production orders ----
    # B-level set-cover greedy (var-blind): pop the B group with fewest
    # un-activated A refs; the activation order gives each A group a
    # B-driven first-use rank fu.  Production order sorts on
    # t* = max(encode-readiness floor, fu): encode readiness only
    # constrains the early part of the stream (the encode finishes ~halfway
    # through the gather stream), after which A rows are produced in the
    # order stage B wants them, so B chunks need only short tA prefixes.
    enc_done_ns = np.cumsum([P * s * ENC_NS_PER_VAR for s in SE_LIST])
    pmin_c = np.maximum(0.0, (enc_done_ns - 2500.0) / GROUP_NS)
    for q, pq in enumerate(per_q):
        n1 = pq["used1"].size
        avs = a_vars[q]
        loc = np.searchsorted(pq["used1"], pq["bq"]).reshape(OUT_PER_Q, -1)
        b_sets = [np.unique(r) for r in loc]
        a2b = [[] for _ in range(n1)]
        for b, refs in enumerate(b_sets):
            for a in refs:
                a2b[a].append(b)
        rem = np.array([len(s) for s in b_sets], dtype=np.int64)
        act = np.zeros(n1, dtype=bool)
        bdone = np.zeros(OUT_PER_Q, dtype=bool)
        heap2 = [(int(rem[b]), b) for b in range(OUT_PER_Q)]
        heapq.heapify(heap2)
        fu = np.empty(n1, dtype=np.int64)
        rank = 0
        while heap2:
            c, b = heapq.heappop(heap2)
            if bdone[b] or c != rem[b]:
                continue
            bdone[b] = True
            for a in b_sets[b]:
                if not act[a]:
                    act[a] = True
                    fu[a] = rank
                    rank += 1
                    for b2 in a2b[a]:
                        if not bdone[b2]:
                            rem[b2] -= 1
                            heapq.heappush(heap2, (int(rem[b2]), b2))
        rpos = np.array(
            [pi[vs].max() if vs.size else 0 for vs in avs], dtype=np.int64
        )
        rchunk = np.searchsorted(VAR_BOUNDS, rpos, side="right")
        pmin = pmin_c[np.minimum(rchunk, ENC_CHUNKS - 1)]
        # list scheduling: fill production slot t with the encode-feasible
        # (pmin <= t) group stage B wants first; if none is feasible yet,
        # fall back to the next-feasible group.
        bypmin = np.argsort(pmin, kind="stable")
        order = np.empty(n1, dtype=np.int64)
        avail = []
        i = 0
        for t in range(n1):
            while i < n1 and pmin[bypmin[i]] <= t:
                heapq.heappush(avail, (int(fu[bypmin[i]]), int(bypmin[i])))
                i += 1
            if avail:
                order[t] = heapq.heappop(avail)[1]
            else:
                order[t] = bypmin[i]
                i += 1
        pq["a_order"] = order
        prod_rank = np.empty(n1, dtype=np.int64)
        prod_rank[pq["a_order"]] = np.arange(n1)
        bkey = prod_rank[loc].max(axis=1)
        pq["b_order"] = np.argsort(bkey, kind="stable")
    return pi, inv


def _pad_groups(n):
    return -(-n // P) * P


def _chunk_group_counts(n_groups, tail_ramp):
    """Groups per dma_gather chunk (FE edges per group). Sizes ramp up at
    the start (small first chunk -> quick desc-gen once the source prefix
    lands) and down at the end (short compute tail -> the last store lands
    early and the consumer unblocks sooner)."""
    g = P
    rem = n_groups
    tail = []
    for s in tail_ramp:
        if rem >= s + g:
            tail.append(s)
            rem -= s
    head = []
    for s in (128, 256):
        if rem >= s + g:
            head.append(s)
            rem -= s
    mid = []
    while rem > 0:
        s = min(GPC, rem)
        mid.append(s)
        rem -= s
    return head + mid + tail[::-1]


def plan(ptrs_list):
    """Partition output groups into quarters, prune each quarter's DAG
    bottom-up, fuse layer pairs, readiness-order stage A.

    Returns (stageA, stageB) dicts with common (padded) shape metadata and
    per-quarter index data:
      n_groups: padded group count, common across quarters
      n_src_rows: rows of the gathered table (A: TAB0, B: nA)
      chunks: list of (n_groups_in_chunk, src_prefix_rows); prefixes are
              maxed over quarters so one compiled program serves all cores
      edge_src: per-quarter list of per-edge source rows, production order
    """
    p0, p1, p2, p3 = [np.asarray(p).astype(np.int64) for p in ptrs_list]
    b_l2 = p3.reshape(-1, 2)  # [2048, 2] L2 group ids
    b_src1 = p2.reshape(-1, 4)[b_l2]  # [2048, 2, 4] L1 ids per out group
    # Quarter assignment: sort out-groups by min L1 reference so groups
    # sharing stage-A work land in the same quarter (cuts ~11% of the
    # cross-quarter duplication vs contiguous quarters).
    qorder = np.argsort(b_src1.reshape(OUT_SIZES[3], -1).min(axis=1), kind="stable")
    quarters = [qorder[OUT_PER_Q * q : OUT_PER_Q * (q + 1)] for q in range(NQ)]

    per_q = []
    for q in range(NQ):
        bq = b_src1[quarters[q]]  # [512, 2, 4]
        used1 = np.unique(bq)
        a_l0 = p1.reshape(-1, 2)[used1]  # [n1, 2] L0 group ids
        a_src0 = p0.reshape(-1, 4)[a_l0]  # [n1, 2, 4] t0 rows
        per_q.append({"used1": used1, "a_src0": a_src0, "bq": bq, "ids": quarters[q]})

    # Joint greedy: B pop order -> A activation order -> var placement.
    # Remap t0 rows through the var order (row 2+2u+k of the reference
    # table lands at 2+2*pi[u]+k; rows 0/1 fixed).
    pi, var_inv = _joint_greedy(per_q)
    rowmap = np.empty(TAB0, dtype=np.int64)
    rowmap[0], rowmap[1] = 0, 1
    u = np.arange(N_VARS, dtype=np.int64)
    rowmap[2 + 2 * u] = 2 + 2 * pi
    rowmap[3 + 2 * u] = 3 + 2 * pi
    for pq in per_q:
        pq["a_src0"] = rowmap[pq["a_src0"]]

    nA = max(_pad_groups(pq["used1"].size) for pq in per_q)

    for pq in per_q:
        n1 = pq["used1"].size
        npad = nA - n1
        # padding groups read only row 0: ready before any encode chunk, so
        # putting them first lets chunk 0's desc-gen+gather warm up under
        # the encode instead of idling the DMA engines.  Real groups follow
        # in activation order, so production row t only needs the var
        # prefix placed by the greedy before activating group t.
        srcA = np.zeros((nA, FE), dtype=np.int64)
        srcA[npad:] = pq["a_src0"].reshape(n1, FE)[pq["a_order"]]
        pq["srcA"] = srcA
        prod = np.empty(n1, dtype=np.int64)
        prod[pq["a_order"]] = npad + np.arange(n1)  # A group id -> prod row

        relabel1 = prod[np.searchsorted(pq["used1"], pq["bq"])]  # [512, 2, 4]
        # B rows in pop order: chunk k of stage B then only needs the tA
        # prefix activated by the greedy up to that point.  The host
        # unpermutes output rows after the run.
        pq["srcB"] = relabel1.reshape(-1, FE)[pq["b_order"]]
        prodB = np.empty(OUT_PER_Q, dtype=np.int64)
        prodB[pq["b_order"]] = np.arange(OUT_PER_Q)  # local g -> produced row
        pq["out_prod"] = prodB

    def mk(srcs, n_src_rows, tail_ramp):
        n_groups = srcs[0].shape[0]
        chunks = []
        g_off = 0
        for gc in _chunk_group_counts(n_groups, tail_ramp):
            m = max(int(s[g_off : g_off + gc].max()) + 1 for s in srcs)
            chunks.append((gc, m))
            g_off += gc
        return {
            "n_groups": n_groups,
            "n_src_rows": n_src_rows,
            "chunks": chunks,
            "edge_src": [s.ravel() for s in srcs],
            "edge_src_2d": [s.reshape(n_groups, FE) for s in srcs],
        }

    stages = [
        mk([pq["srcA"] for pq in per_q], TAB0, (128, 128, 128, 256)),
        mk([pq["srcB"] for pq in per_q], nA, (128, 128)),
    ]
    # stage A production row -> SBUF token id (tA lives in SBUF: row at
    # partition p, rank blk; see build_nc).  Stage B gathers by token, and
    # its chunk prefixes become rank prefixes of the tA tile.
    tok = np.empty(nA, dtype=np.int64)
    g_off = 0
    for gc, _m in stages[0]["chunks"]:
        csub = gc // P
        r = np.arange(gc)
        tok[g_off + r] = 128 * (g_off // P + r % csub) + r // csub
        g_off += gc
    stages[1]["tok_of_row"] = tok
    stages[1]["chunks"] = [
        (gc, max(int(tok[s[o : o + gc]].max()) + 1 for s in stages[1]["edge_src_2d"]))
        for (gc, _m), o in zip(
            stages[1]["chunks"],
            np.cumsum([0] + [gc for gc, _ in stages[1]["chunks"]][:-1]),
        )
    ]
    stages[1]["out_prod"] = [pq["out_prod"] for pq in per_q]
    stages[1]["out_ids"] = [pq["ids"] for pq in per_q]
    stages[0]["var_inv"] = var_inv  # x row order for the device table
    stages[0]["direct"] = None
    stages[1]["direct"] = None
    return stages


def resolve_direct(stages, x):
    """Exact host-side bound check for the direct-form lse (e^arg must stay
    a normal f32), evaluated over all quarters."""
    x = np.asarray(x, dtype=np.float64)[stages[0]["var_inv"]]
    pos = x
    neg = np.log1p(-np.exp(x))
    t0 = np.empty((TAB0, x.shape[1]))
    t0[0] = 0.0
    t0[1] = 0.0
    t0[2::2] = pos
    t0[3::2] = neg
    mn_a = 0.0
    mn_b = 0.0
    for q in range(NQ):
        srcA = stages[0]["edge_src"][q].reshape(-1, 2, 4)
        qv = t0[srcA].sum(axis=2)  # [nA, 2, cols]
        m = qv.max(axis=1)
        lo = qv.min(axis=1)
        tA = m + np.log1p(np.exp(lo - m))
        srcB = stages[1]["edge_src"][q].reshape(-1, 2, 4)
        tb = tA[srcB].sum(axis=2)
        mn_a = min(mn_a, qv.min())
        mn_b = min(mn_b, tb.min())
    stages[0]["direct"] = bool(mn_a > -80.0)
    stages[1]["direct"] = bool(mn_b > -80.0)


def reorder_wrap(stage, q):
    """Permute quarter q's per-edge source ids into dma_gather order and
    wrap into the int16 [128, n_edges//16] SBUF layout (position j ->
    [j%16, j//16], replicated across the 8 gpsimd cores' 16-partition
    groups).

    Edge position j of chunk ci maps to partition p=j%128, slot=j//128,
    cc=slot//FE, k=slot%FE, production row = base_ci + p*csub + cc."""
    src = stage["edge_src"][q]
    tok = stage.get("tok_of_row")
    out = np.empty(stage["n_groups"] * FE, dtype=np.int64)
    base = 0
    e_off = 0
    for gc, _m in stage["chunks"]:
        n_e = gc * FE
        if tok is None:
            # stage A: row-major non-transpose gather; edge position j ->
            # partition j%128, slot j//128, production row base+p*csub+cc
            csub = gc // P
            j = np.arange(n_e)
            p = j % P
            slot = j // P
            cc = slot // FE
            k = slot % FE
            row = base + p * csub + cc
            out[e_off : e_off + n_e] = src[row * FE + k]
        else:
            # stage B: SBUF transpose gather; position i = (j*4+k)*gc + g
            # so the quad adds see contiguous g-blocks; values are tokens
            i = np.arange(n_e)
            jk = i // gc
            g = i % gc
            out[e_off : e_off + n_e] = tok[src[(base + g) * FE + jk]]
        base += gc
        e_off += n_e
    assert out.max() < 2**15 and out.min() >= 0
    out = out.astype(np.int16)
    return np.ascontiguousarray(np.tile(out.reshape(-1, 16).T, (8, 1)))


def build_nc(meta):
    """meta: per-stage (n_groups, n_src_rows, chunks-tuple, direct)."""
    import concourse.bacc as bacc
    import concourse.mybir as mybir
    import concourse.tile as tile

    f32 = mybir.dt.float32
    f16 = mybir.dt.float16
    i16 = mybir.dt.int16
    Alu = mybir.AluOpType
    Act = mybir.ActivationFunctionType

    specs = [
        {"n_groups": n, "n_src_rows": s, "chunks": ch, "direct": dr}
        for (n, s, ch, dr) in meta
    ]

    nc = bacc.Bacc("TRN2", target_bir_lowering=False, debug=False)
    x = nc.dram_tensor("x", [P, S_ENC * B], f16, kind="ExternalInput")
    idx_in = [
        nc.dram_tensor(
            f"idx{l}", [P, FE * s["n_groups"] // 16], i16, kind="ExternalInput"
        )
        for l, s in enumerate(specs)
    ]
    # col-major output: partition p holds batch cols p and 128+p; the host
    # transposes after the run (stage B's SBUF transpose-gather produces
    # column-major data, and a col-major store keeps 512B+ runs)
    out = nc.dram_tensor("out", [P, 2 * OUT_PER_Q], f32, kind="ExternalOutput")

    with tile.TileContext(nc) as tc:
        with (
            tc.tile_pool(name="dram", bufs=1, space="DRAM") as dpool,
            tc.tile_pool(name="sb", bufs=4) as gp,
            tc.tile_pool(name="enc", bufs=5) as ep,
            tc.tile_pool(name="hb", bufs=4) as hp,
            tc.tile_pool(name="tmp", bufs=3) as tp,
            tc.tile_pool(name="ix", bufs=1) as ixp,
        ):
            t0 = dpool.tile([specs[0]["n_src_rows"], B], f16, name="t0", tag="t0")
            # tA lives in SBUF: production row r of chunk ci sits at
            # partition (r-g_off)//csub, rank g_off//128 + (r-g_off)%csub
            # (= token id 128*rank + partition); stage A's lse writes land
            # here directly and stage B token-gathers from it - no DRAM
            # round trip and no store semaphores on the A->B path.
            tA = ixp.tile([P, specs[0]["n_groups"] // P, B], f16, tag="tA")

            # Preload the combined Exp+Ln activation table once; the
            # insert_act_table_loads pass then finds every Exp/Ln already
            # covered and inserts no per-instruction reloads (1283ns each).
            ACT_SET_LN_EXP = 6  # natural_log_exp_and_others
            nc.scalar.add_instruction(
                mybir.InstLoadActFuncSet(
                    name=nc.get_next_instruction_name(),
                    ins=[],
                    outs=[],
                    act_func_set_id=ACT_SET_LN_EXP,
                )
            )

            # table0 rows 0 (-inf in the reference, never gathered) and 1
            # (zeros). Store first so the row prefix [0,2) is ready.
            z = ixp.tile([2, B], f16, tag="z")
            nc.vector.memset(z[:], 0.0)
            nc.sync.dma_start(t0[:][0:2, :], z[:])

            # --- encode, chunked: var v sits at partition (v%1024)//SE,
            # slot v%SE of chunk v//1024; pos row 2+2v, neg row 3+2v.
            # Chunk j covers rows [2+2048j, 2+2048(j+1)), a row prefix, so
            # stage-A gather chunks can start before the whole encode
            # finishes.  Within a chunk each partition's 2*SE rows are
            # contiguous, so the fp16 store is 128 runs of 8KB (full-rate
            # DMA; interleaved layouts would pay the sub-512B 2x penalty).
            # All independent loads are emitted before any store so the
            # in-order DMA queue never has a compute-gated store blocking a
            # ready load: x chunks first, then the index lists.
            # Loads and stores interleave on the in-order DMA queue: keep
            # the x loads two chunks ahead of the encode stores so chunk
            # j's store enters the queue early (first gathers depend on it)
            # while later loads still prefetch under the encode compute.
            xv = x[:].rearrange("p (s b) -> p s b", b=B)
            offs = [0]
            for se_j in SE_LIST:
                offs.append(offs[-1] + se_j)
            xls = []

            def load_x(j):
                se_j = SE_LIST[j]
                xl = ep.tile([P, se_j, B], f16, tag="xl")
                nc.sync.dma_start(xl[:], xv[:, offs[j] : offs[j] + se_j, :])
                xls.append(xl)

            load_x(0)
            ix_t = []
            for l, s in enumerate(specs):
                t = ixp.tile([P, FE * s["n_groups"] // 16], i16, tag=f"ix{l}")
                nc.sync.dma_start(t[:], idx_in[l][:])
                ix_t.append(t)
            load_x(1)
            for j, se_j in enumerate(SE_LIST):
                if j + 2 < ENC_CHUNKS:
                    load_x(j + 2)
                xl = xls[j]
                iv = ep.tile([P, se_j, 2, B], f16, tag="enc")
                et = hp.tile([P, se_j, B], f32, tag="h")
                # pos copy rides the idle DVE so ACT only runs exp+ln
                nc.vector.tensor_scalar_add(iv[:][:, :, 0, :], xl[:], 0.0)
                nc.scalar.activation(et[:], xl[:], Act.Exp)
                nc.scalar.activation(
                    iv[:][:, :, 1, :], et[:], Act.Ln, scale=-1.0, bias=1.0
                )
                r0 = 2 + 2 * P * offs[j]
                r1 = 2 + 2 * P * offs[j + 1]
                # row = r0 + 2*se*p + 2*s + k
                nc.sync.dma_start(
                    t0[:][r0:r1, :].rearrange(
                        "(p s k) b -> p s k b", s=se_j, k=2
                    ),
                    iv[:],
                )

            # --- stage A: gather t0 rows, product-sum, lse into SBUF tA ---
            s = specs[0]
            g_off = 0
            e_off = 0
            for gc, m_src in s["chunks"]:
                csub = gc // P
                ch = gc * FE
                g = gp.tile([P, ch // P, B], f16, tag="g")
                nc.gpsimd.dma_gather(
                    g[:],
                    t0[:][0:m_src, :],
                    ix_t[0][:, e_off // 16 : (e_off + ch) // 16],
                    ch,
                    ch,
                    B,
                    single_packet=False,
                )
                # [p, group, pair(2), fanin(4), b]
                v = g[:].rearrange("p (c j k) b -> p c j k b", j=2, k=4)
                s01 = tp.tile([P, csub, 2, B], f16, tag="m")
                s23 = tp.tile([P, csub, 2, B], f16, tag="n")
                ss = gp.tile([P, csub, 2, B], f16, tag="s")
                nc.vector.tensor_add(s01[:], v[:, :, :, 0, :], v[:, :, :, 1, :])
                nc.vector.tensor_add(s23[:], v[:, :, :, 2, :], v[:, :, :, 3, :])
                nc.vector.tensor_add(ss[:], s01[:], s23[:])
                a = ss[:][:, :, 0, :]
                b = ss[:][:, :, 1, :]
                h = tA[:][:, g_off // P : g_off // P + csub, :]
                if s["direct"]:
                    # lse(a,b) = ln(e^a + e^b): host verified e^min is a
                    # normal f32 (no scaling needed). 1 DVE op; the
                    # whole-tile Exp and the Ln ride the ACT engine.
                    e = tp.tile([P, csub, 2, B], f32, tag="d")
                    d = tp.tile([P, csub, B], f32, tag="sp")
                    nc.scalar.activation(e[:], ss[:], Act.Exp)
                    nc.vector.tensor_add(d[:], e[:][:, :, 0, :], e[:][:, :, 1, :])
                    nc.scalar.activation(h, d[:], Act.Ln)
                else:
                    # wider range: logsumexp = max + ln(1+exp(min-max))
                    m = tp.tile([P, csub, B], f32, tag="d")
                    mn = tp.tile([P, csub, B], f32, tag="sp")
                    sp = tp.tile([P, csub, B], f32, tag="sq")
                    nc.vector.tensor_tensor(m[:], a, b, op=Alu.max)
                    nc.vector.tensor_tensor(mn[:], a, b, op=Alu.min)
                    nc.vector.tensor_tensor(mn[:], mn[:], m[:], op=Alu.subtract)
                    nc.scalar.activation(mn[:], mn[:], Act.Exp)
                    nc.scalar.activation(sp[:], mn[:], Act.Ln, bias=1.0)
                    nc.vector.tensor_add(h, m[:], sp[:])
                g_off += P * csub
                e_off += ch

            # --- stage B: SBUF transpose-gather of tA tokens, reduce, out ---
            s = specs[1]
            ov = out[:].rearrange("p (c g) -> p c g", c=2)
            g_off = 0
            e_off = 0
            for gc, m_tok in s["chunks"]:
                ch = gc * FE
                rank_hi = -(-m_tok // P)
                g = gp.tile([P, 2, ch], f16, tag="g")
                nc.gpsimd.dma_gather(
                    g[:],
                    tA[:][:, 0:rank_hi, :],
                    ix_t[1][:, e_off // 16 : (e_off + ch) // 16],
                    ch,
                    ch,
                    B,
                    transpose=True,
                    single_packet=False,
                    sbuf_tokens_per_rank=P,
                    sbuf_free_dim_per_rank=B * 2,
                )
                # [p, colpair, pair(2), fanin(4), group]
                v = g[:].rearrange("p c (j k g) -> p c j k g", j=2, k=4)
                s01 = tp.tile([P, 2, 2, gc], f16, tag="m")
                s23 = tp.tile([P, 2, 2, gc], f16, tag="n")
                ss = gp.tile([P, 2, 2, gc], f16, tag="s")
                nc.vector.tensor_add(s01[:], v[:, :, :, 0, :], v[:, :, :, 1, :])
                nc.vector.tensor_add(s23[:], v[:, :, :, 2, :], v[:, :, :, 3, :])
                nc.vector.tensor_add(ss[:], s01[:], s23[:])
                h = hp.tile([P, 2, gc], f32, tag="h")
                if s["direct"]:
                    e = tp.tile([P, 2, 2, gc], f32, tag="d")
                    d = tp.tile([P, 2, gc], f32, tag="sp")
                    nc.scalar.activation(e[:], ss[:], Act.Exp)
                    nc.vector.tensor_add(d[:], e[:][:, :, 0, :], e[:][:, :, 1, :])
                    nc.scalar.activation(h[:], d[:], Act.Ln)
                else:
                    a = ss[:][:, :, 0, :]
                    b = ss[:][:, :, 1, :]
                    m = tp.tile([P, 2, gc], f32, tag="d")
                    mn = tp.tile([P, 2, gc], f32, tag="sp")
                    sp = tp.tile([P, 2, gc], f32, tag="sq")
                    nc.vector.tensor_tensor(m[:], a, b, op=Alu.max)
                    nc.vector.tensor_tensor(mn[:], a, b, op=Alu.min)
                    nc.vector.tensor_tensor(mn[:], mn[:], m[:], op=Alu.subtract)
                    nc.scalar.activation(mn[:], mn[:], Act.Exp)
                    nc.scalar.activation(sp[:], mn[:], Act.Ln, bias=1.0)
                    nc.vector.tensor_add(h[:], m[:], sp[:])
                nc.sync.dma_start(ov[:, :, g_off : g_off + gc], h[:])
                g_off += gc
                e_off += ch
    nc.compile()
    return nc


def host_prep(x, ptrs_list, seg_list, stages=None):
    """Host-side sharding + pruning + index preprocessing -> per-core maps."""
    x = np.asarray(x, dtype=np.float32)
    for l, (n_out, f) in enumerate(zip(OUT_SIZES, FANINS)):
        seg = np.asarray(seg_list[l]).astype(np.int64)
        expected = np.repeat(np.arange(n_out, dtype=np.int64), f)
        assert np.array_equal(seg, expected), f"layer {l}: non-uniform segments"

    if stages is None:
        stages = plan(ptrs_list)
    idx_maps = [
        {f"idx{l}": reorder_wrap(s, q) for l, s in enumerate(stages)}
        for q in range(NQ)
    ]

    xvs = []
    xp = x[stages[0]["var_inv"]]  # device var order (greedy placement)
    for cb in range(NCOLB):
        xs = xp[:, cb * B : (cb + 1) * B].astype(np.float16)
        # partition p, slot offs_j+s holds var 128*offs_j + se_j*p + s
        # (partition-major within each encode chunk; see build_nc)
        xv = np.empty((P, S_ENC, B), dtype=np.float16)
        o = 0
        for se_j in SE_LIST:
            xv[:, o : o + se_j] = xs[P * o : P * (o + se_j)].reshape(P, se_j, B)
            o += se_j
        xvs.append(np.ascontiguousarray(xv.reshape(P, -1)))
    # core i -> column block i % NCOLB, quarter i // NCOLB
    return [{"x": xvs[i % NCOLB], **idx_maps[i // NCOLB]} for i in range(NCORES)]


def _meta(stages):
    return tuple(
        (s["n_groups"], s["n_src_rows"], tuple(s["chunks"]), bool(s["direct"]))
        for s in stages
    )


_CACHE = {}


def _get_nc(meta=None):
    if meta is None:
        meta = _CACHE.get("meta")
        assert meta is not None, "call kernel() first"
    if _CACHE.get("meta") != meta:
        _CACHE["nc"] = build_nc(meta)
        _CACHE["meta"] = meta
    return _CACHE["nc"]


def kernel(x, ptrs0, seg0, ptrs1, seg1, ptrs2, seg2, ptrs3, seg3):
    from concourse.bass_utils import run_bass_kernel_spmd

    ptrs_list = [ptrs0, ptrs1, ptrs2, ptrs3]
    stages = plan(ptrs_list)
    resolve_direct(stages, x)
    nc = _get_nc(_meta(stages))
    in_maps = host_prep(x, ptrs_list, [seg0, seg1, seg2, seg3], stages)
    res = run_bass_kernel_spmd(nc, in_maps, core_ids=list(range(NCORES)))
    full = np.empty((OUT_SIZES[3], BATCH), dtype=np.float32)
    for i in range(NCORES):
        cb, q = i % NCOLB, i // NCOLB
        # device out is col-major: [p, c, g_prod] = col c*128+p of prod row
        o = res.results[i]["out"].reshape(P, 2, OUT_PER_Q)
        rows = np.ascontiguousarray(o.transpose(2, 1, 0)).reshape(OUT_PER_Q, B)
        full[stages[1]["out_ids"][q], cb * B : (cb + 1) * B] = rows[
            stages[1]["out_prod"][q]
        ]
    return full
